# revision 4
# baseline (speedup 1.0000x reference)
"""Trainium2 Bass kernel for the Gauss-Seidel-style recurrence

    yMF    = y @ H.T
    s_init = yMF @ Dinv
    s_{k+1} = (yMF - s_k @ U) @ invM        (num_itr iterations)
    returns (s_final, traj=[zeros, s_1, ..., s_num_itr])

Strategy (8 NeuronCores, SPMD):
  Work in the TRANSPOSED state space sT = s.T [N, BS].  Then every matmul
  takes the resident matrix column-block in its NATURAL layout as the
  stationary (lhsT) operand and the thin state as the moving operand:

      tT  = yMFT - U.T  @ sT     -> lhsT = U[:, jb]    (natural columns)
      sT' =        invM.T @ tT   -> lhsT = invM[:, jb] (natural columns)

  Each core owns a 256-row output block jb and keeps U[:,jb], invM[:,jb],
  Dinv[:,jb], H[jb,:].T resident in SBUF (~2MB each).  After each matmul
  group the 256x64 block result is AllGather-ed (64KB/rank, ~5us) so every
  core has the full [2048, 64] state for the next contraction.  No matrix
  ever streams from DRAM inside the loop and there are no on-device
  transposes (host pre-transposes H and y, which is pure data layout).
"""

import sys

sys.path.insert(0, "/opt/trn_rl_repo")

import numpy as np
import concourse.bass as bass
import concourse.mybir as mybir
import concourse.tile as tile
from concourse.bass_utils import run_bass_kernel_spmd

N = 2048
BS = 64
NCORES = 8
JB = N // NCORES          # 256 output rows per core
MT = JB // 128            # 2 output row tiles per core
KT = N // 128             # 16 contraction tiles
f32 = mybir.dt.float32

_cache = {}


def _split_waits(nc, max_waits=4):
    """walrus codegen limits how many semaphore waits one instruction can
    carry (ctrl-encoded Drain/Nop fit only one).  Split overflowing
    instructions into preceding same-engine NOPs, one wait each."""
    for bb in nc.main_func.blocks:
        out = []
        for ins in bb.instructions:
            si = ins.sync_info
            lim = 1
            if si is not None and si.on_wait and len(si.on_wait) > lim:
                waits = list(si.on_wait)
                extra, keep = waits[:-lim], waits[-lim:]
                for w in extra:
                    nop = nc.engines[ins.engine].nop(nofuse=True, hint="waitsplit").ins
                    for b2 in nc.main_func.blocks:
                        if nop in b2.instructions:
                            b2.instructions.remove(nop)
                            break
                    if nop.sync_info is None:
                        nop.sync_info = mybir.SyncInfo(on_wait=[], on_update=[])
                    nop.sync_info.on_wait = [w]
                    out.append(nop)
                si.on_wait = keep
            out.append(ins)
        bb.instructions[:] = out


def _build(num_itr: int):
    nc = bass.Bass("TRN2", target_bir_lowering=False, debug=False, num_devices=NCORES)

    u_blk = nc.dram_tensor("u_blk", [N, JB], f32, kind="ExternalInput")      # U[:, jb]
    m_blk = nc.dram_tensor("m_blk", [N, JB], f32, kind="ExternalInput")      # invM[:, jb]
    d_blk = nc.dram_tensor("d_blk", [N, JB], f32, kind="ExternalInput")      # Dinv[:, jb]
    ht_blk = nc.dram_tensor("ht_blk", [N, JB], f32, kind="ExternalInput")    # H[jb, :].T
    yt = nc.dram_tensor("yt", [N, BS], f32, kind="ExternalInput")            # y.T
    # trajt[i, m, p, b] = s_{i+1}.T[jb_local, b] with local row = 128*m + p
    trajt = nc.dram_tensor("trajt", [num_itr, MT, 128, BS], f32, kind="ExternalOutput")

    rg = [list(range(NCORES))]

    with tile.TileContext(nc) as tc:
        with (
            tc.tile_pool(name="consts", bufs=1) as consts,
            tc.tile_pool(name="state", bufs=2) as state,
            tc.tile_pool(name="own", bufs=3) as own,
            tc.tile_pool(name="psum", bufs=8, space="PSUM") as psum,
            tc.tile_pool(name="dram", bufs=3, space="DRAM") as dram,
        ):
            # ---- load resident matrices as [p, ktile, j] ----
            def load_mat(name, src):
                t = consts.tile([128, KT, JB], f32, tag=name)
                nc.sync.dma_start(t[:], src[:].rearrange("(t p) j -> p t j", p=128))
                return t

            u_sb = load_mat("u_sb", u_blk)
            m_sb = load_mat("m_sb", m_blk)
            d_sb = load_mat("d_sb", d_blk)
            ht_sb = load_mat("ht_sb", ht_blk)

            yt_sb = consts.tile([128, KT, BS], f32)
            nc.sync.dma_start(yt_sb[:], yt[:].rearrange("(t p) b -> p t b", p=128))

            # matmul group: out_own[:, m, :] (+opt sub from ymft) for both m tiles
            def mm_group(lhs_sb, rhs_sb, out_own, sub_from=None):
                for m in range(MT):
                    ps = psum.tile([128, BS], f32, tag="ps")
                    for k in range(KT):
                        nc.tensor.matmul(
                            ps[:],
                            lhs_sb[:, k, m * 128:(m + 1) * 128],
                            rhs_sb[:, k, :],
                            start=(k == 0),
                            stop=(k == KT - 1),
                        )
                    if sub_from is not None:
                        nc.vector.tensor_sub(out_own[:, m, :], sub_from[:, m, :], ps[:])
                    else:
                        nc.vector.tensor_copy(out_own[:, m, :], ps[:])

            # all-gather own block [128, MT, BS] -> full state [128, KT, BS]
            def gather(own_tile, full_tag):
                ag_in = dram.tile([MT, 128, BS], f32, tag="ag_in")
                for m in range(MT):
                    nc.sync.dma_start(ag_in[m], own_tile[:, m, :])
                ag_out = dram.tile([NCORES * MT, 128, BS], f32, tag="ag_out")
                nc.gpsimd.collective_compute(
                    "AllGather",
                    mybir.AluOpType.bypass,
                    replica_groups=rg,
                    ins=[ag_in.opt()],
                    outs=[ag_out.opt()],
                )
                full = state.tile([128, KT, BS], f32, tag=full_tag)
                nc.sync.dma_start(full[:], ag_out[:].rearrange("t p b -> p t b"))
                return full

            # ---- setup: yMFT block, then s_init ----
            ymft_own = consts.tile([128, MT, BS], f32)
            mm_group(ht_sb, yt_sb, ymft_own)
            ymft_full = gather(ymft_own, "st_full")

            st_own = own.tile([128, MT, BS], f32, tag="st_own")
            mm_group(d_sb, ymft_full, st_own)
            st_full = gather(st_own, "st_full")

            # ---- iterations ----
            for i in range(1, num_itr + 1):
                tt_own = own.tile([128, MT, BS], f32, tag="tt_own")
                mm_group(u_sb, st_full, tt_own, sub_from=ymft_own)
                tt_full = gather(tt_own, "tt_full")

                st_own = own.tile([128, MT, BS], f32, tag="st_own")
                mm_group(m_sb, tt_full, st_own)
                for m in range(MT):
                    nc.sync.dma_start(trajt[i - 1, m], st_own[:, m, :])
                if i < num_itr:
                    st_full = gather(st_own, "st_full")

    _split_waits(nc)
    return nc


def _run(num_itr, y, H, Dinv, U, invM, **spmd_kwargs):
    num_itr = int(num_itr)
    if num_itr not in _cache:
        _cache[num_itr] = _build(num_itr)
    nc = _cache[num_itr]

    yt = np.ascontiguousarray(y.T.astype(np.float32, copy=False))
    in_maps = []
    for c in range(NCORES):
        jb = slice(JB * c, JB * (c + 1))
        in_maps.append({
            "u_blk": np.ascontiguousarray(U[:, jb], dtype=np.float32),
            "m_blk": np.ascontiguousarray(invM[:, jb], dtype=np.float32),
            "d_blk": np.ascontiguousarray(Dinv[:, jb], dtype=np.float32),
            "ht_blk": np.ascontiguousarray(H[jb, :].T, dtype=np.float32),
            "yt": yt,
        })
    res = run_bass_kernel_spmd(nc, in_maps, core_ids=list(range(NCORES)), **spmd_kwargs)
    # trajt block from core c: [num_itr, MT, 128, BS]; global row j = JB*c + 128*m + p
    blocks = [res.results[c]["trajt"].reshape(num_itr, JB, BS) for c in range(NCORES)]
    trajT = np.concatenate(blocks, axis=1)                     # [num_itr, N, BS]
    traj_iters = np.transpose(trajT, (0, 2, 1))                # [num_itr, BS, N]
    traj = np.concatenate(
        [np.zeros((1, BS, N), np.float32), traj_iters], axis=0
    )
    traj = np.ascontiguousarray(traj, dtype=np.float32)
    s_final = traj[num_itr].copy()
    return (s_final, traj), res


def kernel(num_itr, bs, y, H, Dinv, U, invM):
    assert int(bs) == BS and y.shape == (BS, N) and H.shape == (N, N)
    out, _ = _run(num_itr, y, H, Dinv, U, invM)
    return out
